# revision 7
# baseline (speedup 1.0000x reference)
"""Trainium2 Bass kernel for per-head L2-distance attention + grouped output
projection + BatchNorm (dense_transformer, B=2, dim=256, N=3072, H=8, D=32).

Sharding: one head per NeuronCore (8 heads = 8 cores), both batches on each
core.  Channels split by head, so the BatchNorm per-channel (b, n) reduction
is fully core-local -> zero collectives.

v3 design (vs v2 @ 225.8us):
  - Same folded math as v2: W = -2 wq^T wk contracts x against g = W^T x;
    augmented K=96 contraction gives ST[j,i] = ||q_i - k_j||^2 in one matmul
    pair; W2 = (wo wv)^T collapses PV + output projection; exp chain is
    ACT Sqrt (t-domain) -> custom DVE cubic+3sq poly (precision-critical:
    BN divides by tiny per-channel variance, ~50x error amplification).
  - DVE exp ops are TRIPLE-wide ([128, 3072] spanning 3 j-tiles): DVE cost
    is free-size-driven, so 48 ops x ~3.3us beat 144 x ~1.2us by ~15us.
    The DVE was the saturated engine (98% busy in-loop).
  - PSUM re-quadrant: one persistent [128, 1024] accumulator tile holds
    outX at partitions 64:97 (PV accumulation, tile_position col 64),
    y = W2@outn at partitions 0:32 (bn_stats source), and the last-block
    rowsum-broadcast at 32:64.  That frees 2 banks -> ps_st bufs=3, so the
    ACT never waits on ST matmul double-buffering (block-boundary stalls
    were ~26us).  Engines handle mismatched in/out base partitions for
    built-in ops (probed on HW); only custom DVE ops are base-0-only.
  - outX -> SBUF epilogue copy moved ACT -> DVE (fits the DVE's block-
    boundary slack; keeps ACT streaming sqrts across block boundaries).
  - ones rows of qa/ka come from a [1, N] DRAM row via partition-stride-0
    broadcast DMA (was 786KB of host-staged ones).
  - b=1 prologue projections are emitted after b=0's first main block so
    their PSUM slots don't gate b=0 loop start.
  - Tail: last block skips the [128,8] reshape (direct [1,1024] bf16
    reciprocal feeds the PE rowsum broadcast); affines alternate ACT/DVE
    3/3; output DMAs alternate queues.
  Known dead ends (measured): fp8-DoubleRow ST (2.2e-2 rel err) and fp8 PV
  (5.7e-2 sim), u-domain DVE-only exp poly (systematic 1.3% -> 7e-2 after
  BN), >512-free matmul (fp32 PSUM out is mandatory on TRN2), bf16 PSUM
  (TRN3-only), DMA cannot touch PSUM, custom DVE 2x perf mode (slice
  budget 2x7>8), exp-on-ACT rebalance (Sqrt and Exp live in different
  1.28us-swap ACT tables), GPSIMD exp (no library; toolchain unavailable),
  bn_stats >512 free (HW limit).
"""

import numpy as np

import concourse.bass as bass
import concourse.tile as tile
from concourse import bacc, mybir
from concourse.bass_utils import run_bass_kernel_spmd

F32 = mybir.dt.float32
F32R = mybir.dt.float32r
BF16 = mybir.dt.bfloat16
AFT = mybir.ActivationFunctionType

B, DIM, N, H, D = 2, 256, 3072, 8, 32
C = DIM // H          # 32 input channels per head
NT = N // 128         # 24 j-tiles
NB = N // 1024        # 3 i-blocks per batch
EPS_BN = 1e-5

# --- exp polynomial calibration (baseline-proven, rel err ~1.5e-6) ---
# w = sqrt(d2)/(8*sqrt(32)); minimax cubic for exp(-t/gam) on t in [0, W*gam],
# rescaled so the cubic coefficient is -1:  p(t) = B0 + t*(B1 + t*(B2 - t)),
# out = ((p^2)^2)^2 = exp(-sqrt(d2)/sqrt(32)).
B0 = 0.999999894052468
B1 = -1.858707805584652
B2 = 1.7242982194980068
ACT_SCALE = 0.00014132731  # (gam*scale/8)^2
ACT_BIAS = 1.413273e-09    # ACT_SCALE * 1e-5 protective epsilon inside sqrt

_EXP_OP = None


def _register_exp_op():
    """Register the exp(-.) polynomial as a custom DVE op (in-process)."""
    global _EXP_OP
    if _EXP_OP is not None:
        return _EXP_OP
    import concourse.dve_ops as dve_ops
    from concourse.dve_spec import Spec, Src0, C0, C1, C2, sq, lower, _has_src1
    from concourse.dve_uop import DveOpSpec

    name = "EXP_NEG_POLY3SQ3_ANT"
    for o in dve_ops.OPS:
        if o.name == name:
            _EXP_OP = o
            return o

    t = Src0
    body = sq(sq(sq(C0 + t * (C1 + t * (C2 - t)))))

    def ref(in0, in1, c0, c1, c2):
        tt = in0.astype(np.float32)
        p = (c0 + tt * (c1 + tt * (c2 - tt))).astype(np.float32)
        for _ in range(3):
            p = (p * p).astype(np.float32)
        return p

    spec = Spec(body=body, reference=ref)
    row = dve_ops._CUSTOM_DVE_ROW_BASE + len(dve_ops.OPS)
    shas = {}
    for ver in ("v3", "v4"):
        try:
            uops = lower(spec, ver=ver)
            s = DveOpSpec(name=name, opcode=row, uops=uops, rd1_en=_has_src1(spec))
            shas[ver] = s.sha(ver)
        except Exception:
            pass
    op = dve_ops.DveOp(name, spec, subdim=False, uops_sha=shas)
    dve_ops.OPS.append(op)
    dve_ops._SUB_OPCODE_FOR_NAME[name] = row
    dve_ops.CUSTOM_DVE_SPECS[name] = spec
    _EXP_OP = op
    return op


def _bcast_rows(ap: bass.AP, nrows: int) -> bass.AP:
    """[1, n] AP -> partition-stride-0 [nrows, n] AP (for DMA replicate)."""
    return bass.AP(tensor=ap.tensor, offset=ap.offset, ap=[[0, nrows], ap.ap[-1]])


def build_program():
    exp_op = _register_exp_op()
    nc = bacc.Bacc("TRN2", target_bir_lowering=False, debug=False)

    xh_d = nc.dram_tensor("xh", [B, C, N], BF16, kind="ExternalInput").ap()
    xt_d = nc.dram_tensor("xt", [B, 128, NT, 33], BF16, kind="ExternalInput").ap()
    on_d = nc.dram_tensor("on1", [1, N], BF16, kind="ExternalInput").ap()
    wall_d = nc.dram_tensor("wall", [C, 96], BF16, kind="ExternalInput").ap()
    w2_d = nc.dram_tensor("w2t", [C, C], F32R, kind="ExternalInput").ap()
    gm_d = nc.dram_tensor("gm", [C, 1], F32, kind="ExternalInput").ap()
    bt_d = nc.dram_tensor("bt", [C, 1], F32, kind="ExternalInput").ap()
    y_d = nc.dram_tensor("y", [B, C, N], F32, kind="ExternalOutput").ap()

    with tile.TileContext(nc) as tc:
        with tc.tile_pool(name="const", bufs=1) as const, \
             tc.tile_pool(name="persist", bufs=1) as persist, \
             tc.tile_pool(name="drp", bufs=2, space="DRAM") as drp:
            wall_s = const.tile([C, 96], BF16)
            w2_s = const.tile([C, C], F32R)
            gm_s = const.tile([C, 1], F32)
            bt_s = const.tile([C, 1], F32)
            actb = const.tile([128, 1], F32)
            epsb = const.tile([C, 1], F32)
            ones1_s = const.tile([1, C], BF16)
            nc.vector.memset(ones1_s, 1.0)
            for dst, src in ((wall_s, wall_d), (w2_s, w2_d),
                             (gm_s, gm_d), (bt_s, bt_d)):
                nc.sync.dma_start(out=dst, in_=src)
            nc.vector.memset(actb, ACT_BIAS)
            nc.vector.memset(epsb, EPS_BN)
            # force the (single) sqrt_and_others table load off the critical
            # path: first ACT op is a Sqrt, so Square ops later reuse the set
            warm = const.tile([1, 1], F32)
            nc.scalar.activation(warm, epsb[0:1, :], AFT.Sqrt,
                                 bias=0.0, scale=1.0)

            outn_sb = [persist.tile([C, N], F32R, tag=f"on{b}", name=f"outn{b}")
                       for b in range(B)]
            stats = persist.tile([C, B * NB * 2, 6], F32)

            with tc.tile_pool(name="xb", bufs=1) as xbp, \
                 tc.tile_pool(name="pproj", bufs=1, space="PSUM") as pprojp, \
                 tc.tile_pool(name="st", bufs=2, space="PSUM") as ps_st, \
                 tc.tile_pool(name="acc", bufs=1, space="PSUM") as ps_acc, \
                 tc.tile_pool(name="mt", bufs=2) as mt, \
                 tc.tile_pool(name="ep", bufs=2) as ep:
                pro = {}
                for b in range(B):
                    qa = xbp.tile([96, N], BF16, tag=f"qa{b}", name=f"qa{b}")
                    ka = xbp.tile([96, N], BF16, tag=f"ka{b}", name=f"ka{b}")
                    xaT = xbp.tile([128, NT, 33], BF16, tag=f"xaT{b}",
                                   name=f"xaT{b}")
                    # x straight into qa's qk-rows; ones rows broadcast from
                    # a [1, N] DRAM row; x^T (with ones col) from host
                    nc.gpsimd.dma_start(out=qa[0:32, 0:1024],
                                        in_=xh_d[b][:, 0:1024])
                    nc.gpsimd.dma_start(out=qa[0:32, 1024:3072],
                                        in_=xh_d[b][:, 1024:3072])
                    nc.sync.dma_start(out=qa[64:96, :], in_=_bcast_rows(on_d, 32))
                    nc.sync.dma_start(out=ka[32:64, :], in_=_bcast_rows(on_d, 32))
                    nc.gpsimd.dma_start(out=xaT, in_=xt_d[b])
                    pro[b] = (qa, ka, xaT)

                def emit_proj(b):
                    # combined projection: rows 0-31 g = W^T x, 32-63 q,
                    # 64-95 k.  g-copy on DVE (custom-op-free, base 0),
                    # squares on ACT.
                    qa, ka, _ = pro[b]
                    for icx in range(NB):
                        sl = bass.ts(icx, 1024)
                        pa = pprojp.tile([96, 1024], F32, tag="pp",
                                         name="pa")
                        for hh in range(2):
                            nc.tensor.matmul(
                                pa[:, bass.ts(hh, 512)], lhsT=wall_s,
                                rhs=qa[0:32, bass.ds(icx * 1024 + hh * 512,
                                                     512)],
                                start=True, stop=True)
                        nc.vector.tensor_copy(ka[0:32, sl], pa[0:32, :])
                        nc.scalar.activation(qa[32:64, sl], pa[32:64, :],
                                             AFT.Square, bias=0.0, scale=1.0)
                        nc.scalar.activation(ka[64:96, sl], pa[64:96, :],
                                             AFT.Square, bias=0.0, scale=1.0)

                def emit_block(b, icb, blk):
                    qa, ka, xaT = pro[b]
                    t3 = None
                    outX_ps = ps_acc.tile([33, 1024], F32, tag="outX",
                                          name="outX_ps")
                    for jt in range(NT):
                        st_t = ps_st.tile([128, 1024], F32, tag="st",
                                          name="st_ps")
                        for hh in range(2):
                            nsl = bass.ds(icb * 1024 + hh * 512, 512)
                            nc.tensor.matmul(
                                st_t[:, bass.ts(hh, 512)],
                                lhsT=ka[:, bass.ts(jt, 128)],
                                rhs=qa[:, nsl],
                                start=True, stop=True)
                        pos = jt % 3
                        if pos == 0:
                            t3 = mt.tile([128, 3 * 1024], F32, tag="t",
                                         name="t3", bufs=2)
                        nc.scalar.activation(
                            t3[:, bass.ts(pos, 1024)], st_t, AFT.Sqrt,
                            bias=actb, scale=ACT_SCALE)
                        if pos == 2:
                            p3 = mt.tile([128, 3 * 1024], BF16, tag="p",
                                         name="p3", bufs=2)
                            nc.vector._custom_dve(exp_op, out=p3, in0=t3,
                                                  s0=B0, s1=B1, imm2=B2)
                            for j in (jt - 2, jt - 1, jt):
                                for hh in range(2):
                                    nc.tensor.matmul(
                                        outX_ps[:, bass.ts(hh, 512)],
                                        lhsT=xaT[:, j, :],
                                        rhs=p3[:, bass.ds((j % 3) * 1024 +
                                                          hh * 512, 512)],
                                        start=(j == 0),
                                        stop=(j == NT - 1))
                    # epilogue: outX -> SBUF on DVE (fits its block-boundary
                    # slack; ACT keeps streaming sqrts), then rowsum recip,
                    # normalize, y = W2@outn for bn_stats.
                    outc = ep.tile([33, 1024], F32, tag="outc", name="outc")
                    nc.vector.tensor_copy(outc, outX_ps)
                    last = (b == B - 1 and icb == NB - 1)
                    osl = bass.ds(icb * 1024, 1024)
                    if not last:
                        r128 = ep.tile([128, 8], F32, tag="r128", name="r128")
                        nc.gpsimd.dma_start(out=r128, in_=outc[32:33, :])
                        recip = ep.tile([128, 8], F32, tag="recip",
                                        name="recip")
                        nc.vector.reciprocal(recip, r128)
                        rdr = drp.tile([1, 1024], F32, tag="rdr", name="rdr")
                        nc.gpsimd.dma_start(out=rdr, in_=recip)
                        rbc = ep.tile([C, 1024], F32, tag="rbc", name="rbc")
                        nc.gpsimd.dma_start(out=rbc, in_=_bcast_rows(rdr, C))
                        nc.gpsimd.tensor_mul(outn_sb[b][:, osl],
                                             outc[0:32, :], rbc)
                    else:
                        # final block: shortest-latency path (direct bf16
                        # reciprocal, PE bcast, DVE mul) - nothing overlaps
                        # the tail
                        rr1 = ep.tile([1, 1024], BF16, tag="rr1", name="rr1")
                        with nc.allow_low_precision(
                                reason="rowsum recip feeds bf16 PE bcast"):
                            nc.vector.reciprocal(rr1, outc[32:33, :])
                    for hh in range(2):
                        hsl = bass.ds(icb * 1024 + hh * 512, 512)
                        if last:
                            rb_ps = pprojp.tile([C, 512], F32, tag="pp",
                                                name="rb_ps")
                            nc.tensor.matmul(rb_ps, lhsT=ones1_s,
                                             rhs=rr1[:, bass.ts(hh, 512)],
                                             start=True, stop=True)
                            nc.vector.tensor_mul(
                                outn_sb[b][:, hsl],
                                outc[0:32, bass.ts(hh, 512)], rb_ps)
                        y_ps = pprojp.tile([C, 512], F32, tag="pp",
                                           name="y_ps")
                        nc.tensor.matmul(y_ps, lhsT=w2_s,
                                         rhs=outn_sb[b][:, hsl],
                                         start=True, stop=True)
                        nc.vector.bn_stats(stats[:, blk * 2 + hh, :], y_ps)

                emit_proj(0)
                emit_block(0, 0, 0)
                emit_proj(1)
                emit_block(0, 1, 1)
                emit_block(0, 2, 2)
                for icb in range(NB):
                    emit_block(1, icb, NB + icb)

            # BatchNorm tail: aggregate stats, re-project, affine, store
            with tc.tile_pool(name="tail", bufs=1) as tail, \
                 tc.tile_pool(name="ps_tl", bufs=2, space="PSUM") as ps_tl:
                mv = tail.tile([C, 2], F32)
                nc.vector.bn_aggr(mv, stats)
                std = tail.tile([C, 1], F32)
                nc.scalar.activation(std, mv[:, 1:2], AFT.Sqrt,
                                     bias=epsb, scale=1.0)
                rstd = tail.tile([C, 1], F32)
                nc.vector.reciprocal(rstd, std)
                sc = tail.tile([C, 1], F32)
                nc.vector.tensor_mul(sc, gm_s, rstd)
                msc = tail.tile([C, 1], F32)
                nc.vector.tensor_mul(msc, mv[:, 0:1], sc)
                nb = tail.tile([C, 1], F32)
                nc.vector.tensor_sub(nb, bt_s, msc)
                k = 0
                for b in range(B):
                    for icx in range(NB):
                        sl = bass.ts(icx, 1024)
                        yt_ps = ps_tl.tile([C, 1024], F32, tag="yt",
                                           name="yt_ps")
                        for hh in range(2):
                            nc.tensor.matmul(
                                yt_ps[:, bass.ts(hh, 512)], lhsT=w2_s,
                                rhs=outn_sb[b][:, bass.ds(icx * 1024 +
                                                          hh * 512, 512)],
                                start=True, stop=True)
                        yo = tail.tile([C, 1024], F32, tag="yo", name="yo",
                                       bufs=4)
                        if k % 2 == 0:
                            nc.scalar.activation(yo, yt_ps, AFT.Identity,
                                                 bias=nb, scale=sc)
                        else:
                            nc.vector.tensor_scalar(
                                out=yo, in0=yt_ps, scalar1=sc, scalar2=nb,
                                op0=mybir.AluOpType.mult,
                                op1=mybir.AluOpType.add)
                        if k % 2 == 0:
                            nc.sync.dma_start(out=y_d[b][:, sl], in_=yo)
                        else:
                            nc.gpsimd.dma_start(out=y_d[b][:, sl], in_=yo)
                        k += 1

    nc.compile()
    return nc


_NC_CACHE = None


def _get_nc():
    global _NC_CACHE
    if _NC_CACHE is None:
        _NC_CACHE = build_program()
    return _NC_CACHE


def make_in_maps(x, wq, wk, wv, wo, gamma, beta):
    import ml_dtypes
    f = np.float32
    bf = ml_dtypes.bfloat16
    ones1 = np.ones((1, N), f).astype(bf)
    in_maps = []
    for h in range(H):
        cs = slice(h * C, (h + 1) * C)
        xh = np.ascontiguousarray(x[:, cs, :]).astype(bf)
        # x^T tiles with ones column: [B, 128, NT, 33]
        xt = np.ones((B, 128, NT, 33), f)
        xtr = np.ascontiguousarray(x[:, cs, :].transpose(0, 2, 1))  # [B,N,C]
        xt[:, :, :, :32] = xtr.reshape(B, NT, 128, C).transpose(0, 2, 1, 3)
        wall = np.empty((C, 96), f)
        wall[:, 0:32] = -2.0 * (wk[h].T @ wq[h])    # g-rows weight (G.T)
        wall[:, 32:64] = wq[h].T
        wall[:, 64:96] = wk[h].T
        in_maps.append({
            "xh": xh,
            "xt": xt.astype(bf),
            "on1": ones1,
            "wall": wall.astype(bf),
            "w2t": np.ascontiguousarray((wo[h] @ wv[h]).T.astype(f)),
            "gm": np.ascontiguousarray(gamma[cs].reshape(C, 1).astype(f)),
            "bt": np.ascontiguousarray(beta[cs].reshape(C, 1).astype(f)),
        })
    return in_maps


def kernel(x, wq, wk, wv, wo, bo, gamma, beta):
    x, wq, wk, wv, wo, gamma, beta = (np.asarray(a) for a in
                                      (x, wq, wk, wv, wo, gamma, beta))
    nc = _get_nc()
    in_maps = make_in_maps(x, wq, wk, wv, wo, gamma, beta)
    res = run_bass_kernel_spmd(nc, in_maps, list(range(H)))
    y = np.empty((B, DIM, N), np.float32)
    for h in range(H):
        y[:, h * C:(h + 1) * C, :] = res.results[h]["y"]
    return y


# revision 8
# speedup vs baseline: 1.3112x; 1.3112x over previous
"""Trainium2 Bass kernel for per-head L2-distance attention + grouped output
projection + BatchNorm (dense_transformer, B=2, dim=256, N=3072, H=8, D=32).

Sharding: one head per NeuronCore (8 heads = 8 cores), both batches on each
core.  Channels split by head, so the BatchNorm per-channel (b, n) reduction
is fully core-local -> zero collectives.

v3 design (vs v2 @ 225.8us):
  - Same folded math as v2: W = -2 wq^T wk contracts x against g = W^T x;
    augmented K=96 contraction gives ST[j,i] = ||q_i - k_j||^2 in one matmul
    pair; W2 = (wo wv)^T collapses PV + output projection; exp chain is
    ACT Sqrt (t-domain) -> custom DVE cubic+3sq poly (precision-critical:
    BN divides by tiny per-channel variance, ~50x error amplification).
  - DVE exp ops are TRIPLE-wide ([128, 3072] spanning 3 j-tiles): DVE cost
    is free-size-driven, so 48 ops x ~3.3us beat 144 x ~1.2us by ~15us.
    The DVE was the saturated engine (98% busy in-loop).
  - PSUM re-quadrant: one persistent [128, 1024] accumulator tile holds
    outX at partitions 64:97 (PV accumulation, tile_position col 64),
    y = W2@outn at partitions 0:32 (bn_stats source), and the last-block
    rowsum-broadcast at 32:64.  That frees 2 banks -> ps_st bufs=3, so the
    ACT never waits on ST matmul double-buffering (block-boundary stalls
    were ~26us).  Engines handle mismatched in/out base partitions for
    built-in ops (probed on HW); only custom DVE ops are base-0-only.
  - outX -> SBUF epilogue copy moved ACT -> DVE (fits the DVE's block-
    boundary slack; keeps ACT streaming sqrts across block boundaries).
  - ones rows of qa/ka come from a [1, N] DRAM row via partition-stride-0
    broadcast DMA (was 786KB of host-staged ones).
  - b=1 prologue projections are emitted after b=0's first main block so
    their PSUM slots don't gate b=0 loop start.
  - Tail: last block skips the [128,8] reshape (direct [1,1024] bf16
    reciprocal feeds the PE rowsum broadcast); affines alternate ACT/DVE
    3/3; output DMAs alternate queues.
  Known dead ends (measured): fp8-DoubleRow ST (2.2e-2 rel err) and fp8 PV
  (5.7e-2 sim), u-domain DVE-only exp poly (systematic 1.3% -> 7e-2 after
  BN), >512-free matmul (fp32 PSUM out is mandatory on TRN2), bf16 PSUM
  (TRN3-only), DMA cannot touch PSUM, custom DVE 2x perf mode (slice
  budget 2x7>8), exp-on-ACT rebalance (Sqrt and Exp live in different
  1.28us-swap ACT tables), GPSIMD exp (no library; toolchain unavailable),
  bn_stats >512 free (HW limit).
"""

import numpy as np

import concourse.bass as bass
import concourse.tile as tile
from concourse import bacc, mybir
from concourse.bass_utils import run_bass_kernel_spmd

F32 = mybir.dt.float32
F32R = mybir.dt.float32r
BF16 = mybir.dt.bfloat16
AFT = mybir.ActivationFunctionType

B, DIM, N, H, D = 2, 256, 3072, 8, 32
C = DIM // H          # 32 input channels per head
NT = N // 128         # 24 j-tiles
NB = N // 1024        # 3 i-blocks per batch
EPS_BN = 1e-5

# --- exp polynomial calibration (baseline-proven, rel err ~1.5e-6) ---
# w = sqrt(d2)/(8*sqrt(32)); minimax cubic for exp(-t/gam) on t in [0, W*gam],
# rescaled so the cubic coefficient is -1:  p(t) = B0 + t*(B1 + t*(B2 - t)),
# out = ((p^2)^2)^2 = exp(-sqrt(d2)/sqrt(32)).
B0 = 0.999999894052468
B1 = -1.858707805584652
B2 = 1.7242982194980068
ACT_SCALE = 0.00014132731  # (gam*scale/8)^2
ACT_BIAS = 1.413273e-09    # ACT_SCALE * 1e-5 protective epsilon inside sqrt

_EXP_OP = None


def _register_exp_op():
    """Register the exp(-.) polynomial as a custom DVE op (in-process)."""
    global _EXP_OP
    if _EXP_OP is not None:
        return _EXP_OP
    import concourse.dve_ops as dve_ops
    from concourse.dve_spec import Spec, Src0, C0, C1, C2, sq, lower, _has_src1
    from concourse.dve_uop import DveOpSpec

    name = "EXP_NEG_POLY3SQ3_ANT"
    for o in dve_ops.OPS:
        if o.name == name:
            _EXP_OP = o
            return o

    t = Src0
    body = sq(sq(sq(C0 + t * (C1 + t * (C2 - t)))))

    def ref(in0, in1, c0, c1, c2):
        tt = in0.astype(np.float32)
        p = (c0 + tt * (c1 + tt * (c2 - tt))).astype(np.float32)
        for _ in range(3):
            p = (p * p).astype(np.float32)
        return p

    spec = Spec(body=body, reference=ref)
    row = dve_ops._CUSTOM_DVE_ROW_BASE + len(dve_ops.OPS)
    shas = {}
    for ver in ("v3", "v4"):
        try:
            uops = lower(spec, ver=ver)
            s = DveOpSpec(name=name, opcode=row, uops=uops, rd1_en=_has_src1(spec))
            shas[ver] = s.sha(ver)
        except Exception:
            pass
    op = dve_ops.DveOp(name, spec, subdim=False, uops_sha=shas)
    dve_ops.OPS.append(op)
    dve_ops._SUB_OPCODE_FOR_NAME[name] = row
    dve_ops.CUSTOM_DVE_SPECS[name] = spec
    _EXP_OP = op
    return op


def _bcast_rows(ap: bass.AP, nrows: int) -> bass.AP:
    """[1, n] AP -> partition-stride-0 [nrows, n] AP (for DMA replicate)."""
    return bass.AP(tensor=ap.tensor, offset=ap.offset, ap=[[0, nrows], ap.ap[-1]])


def build_program():
    exp_op = _register_exp_op()
    nc = bacc.Bacc("TRN2", target_bir_lowering=False, debug=False)

    xh_d = nc.dram_tensor("xh", [B, C, N], BF16, kind="ExternalInput").ap()
    xt_d = nc.dram_tensor("xt", [B, 128, NT, 33], BF16, kind="ExternalInput").ap()
    on_d = nc.dram_tensor("on1", [1, N], BF16, kind="ExternalInput").ap()
    wall_d = nc.dram_tensor("wall", [C, 96], BF16, kind="ExternalInput").ap()
    w2_d = nc.dram_tensor("w2t", [C, C], F32R, kind="ExternalInput").ap()
    gm_d = nc.dram_tensor("gm", [C, 1], F32, kind="ExternalInput").ap()
    bt_d = nc.dram_tensor("bt", [C, 1], F32, kind="ExternalInput").ap()
    y_d = nc.dram_tensor("y", [B, C, N], F32, kind="ExternalOutput").ap()

    with tile.TileContext(nc) as tc:
        with tc.tile_pool(name="const", bufs=1) as const, \
             tc.tile_pool(name="persist", bufs=1) as persist, \
             tc.tile_pool(name="drp", bufs=2, space="DRAM") as drp:
            wall_s = const.tile([C, 96], BF16)
            w2_s = const.tile([C, C], F32R)
            gm_s = const.tile([C, 1], F32)
            bt_s = const.tile([C, 1], F32)
            actb = const.tile([128, 1], F32)
            epsb = const.tile([C, 1], F32)
            ones1_s = const.tile([1, C], BF16)
            nc.vector.memset(ones1_s, 1.0)
            for dst, src in ((wall_s, wall_d), (w2_s, w2_d),
                             (gm_s, gm_d), (bt_s, bt_d)):
                nc.sync.dma_start(out=dst, in_=src)
            nc.vector.memset(actb, ACT_BIAS)
            nc.vector.memset(epsb, EPS_BN)
            # force the (single) sqrt_and_others table load off the critical
            # path: first ACT op is a Sqrt, so Square ops later reuse the set
            warm = const.tile([1, 1], F32)
            nc.scalar.activation(warm, epsb[0:1, :], AFT.Sqrt,
                                 bias=0.0, scale=1.0)

            outn_sb = [persist.tile([C, N], F32R, tag=f"on{b}", name=f"outn{b}")
                       for b in range(B)]
            stats = persist.tile([C, B * NB * 2, 6], F32)

            with tc.tile_pool(name="xb", bufs=1) as xbp, \
                 tc.tile_pool(name="pproj", bufs=1, space="PSUM") as pprojp, \
                 tc.tile_pool(name="st", bufs=2, space="PSUM") as ps_st, \
                 tc.tile_pool(name="acc", bufs=1, space="PSUM") as ps_acc, \
                 tc.tile_pool(name="mt", bufs=2) as mt, \
                 tc.tile_pool(name="ep", bufs=2) as ep:
                pro = {}
                for b in range(B):
                    qa = xbp.tile([96, N], BF16, tag=f"qa{b}", name=f"qa{b}")
                    ka = xbp.tile([96, N], BF16, tag=f"ka{b}", name=f"ka{b}")
                    xaT = xbp.tile([128, NT, 33], BF16, tag=f"xaT{b}",
                                   name=f"xaT{b}")
                    # x straight into qa's qk-rows; ones rows broadcast from
                    # a [1, N] DRAM row; x^T (with ones col) from host
                    nc.gpsimd.dma_start(out=qa[0:32, 0:1024],
                                        in_=xh_d[b][:, 0:1024])
                    nc.gpsimd.dma_start(out=qa[0:32, 1024:3072],
                                        in_=xh_d[b][:, 1024:3072])
                    nc.sync.dma_start(out=qa[64:96, :], in_=_bcast_rows(on_d, 32))
                    nc.sync.dma_start(out=ka[32:64, :], in_=_bcast_rows(on_d, 32))
                    nc.gpsimd.dma_start(out=xaT, in_=xt_d[b])
                    pro[b] = (qa, ka, xaT)

                def emit_proj(b):
                    # combined projection: rows 0-31 g = W^T x, 32-63 q,
                    # 64-95 k.  g-copy on DVE (custom-op-free, base 0),
                    # squares on ACT.
                    qa, ka, _ = pro[b]
                    for icx in range(NB):
                        sl = bass.ts(icx, 1024)
                        pa = pprojp.tile([96, 1024], F32, tag="pp",
                                         name="pa")
                        for hh in range(2):
                            nc.tensor.matmul(
                                pa[:, bass.ts(hh, 512)], lhsT=wall_s,
                                rhs=qa[0:32, bass.ds(icx * 1024 + hh * 512,
                                                     512)],
                                start=True, stop=True)
                        nc.vector.tensor_copy(ka[0:32, sl], pa[0:32, :])
                        nc.scalar.activation(qa[32:64, sl], pa[32:64, :],
                                             AFT.Square, bias=0.0, scale=1.0)
                        nc.scalar.activation(ka[64:96, sl], pa[64:96, :],
                                             AFT.Square, bias=0.0, scale=1.0)

                def emit_block(b, icb, blk):
                    qa, ka, xaT = pro[b]
                    outX_ps = ps_acc.tile([33, 1024], F32, tag="outX",
                                          name="outX_ps")
                    for jt in range(NT):
                        st_t = ps_st.tile([128, 1024], F32, tag="st",
                                          name="st_ps")
                        for hh in range(2):
                            nsl = bass.ds(icb * 1024 + hh * 512, 512)
                            nc.tensor.matmul(
                                st_t[:, bass.ts(hh, 512)],
                                lhsT=ka[:, bass.ts(jt, 128)],
                                rhs=qa[:, nsl],
                                start=True, stop=True)
                        t_sb = mt.tile([128, 1024], F32, tag="t",
                                       name="t_sb", bufs=4)
                        nc.scalar.activation(t_sb, st_t, AFT.Sqrt,
                                             bias=actb, scale=ACT_SCALE)
                        p_sb = mt.tile([128, 1024], BF16, tag="p",
                                       name="p_sb", bufs=4)
                        nc.vector._custom_dve(exp_op, out=p_sb, in0=t_sb,
                                              s0=B0, s1=B1, imm2=B2)
                        for hh in range(2):
                            nc.tensor.matmul(
                                outX_ps[:, bass.ts(hh, 512)],
                                lhsT=xaT[:, jt, :],
                                rhs=p_sb[:, bass.ts(hh, 512)],
                                start=(jt == 0),
                                stop=(jt == NT - 1))
                    # epilogue: one ACT pass brings outX (incl rowsum row)
                    # to SBUF, then rowsum recip, normalize, y = W2@outn
                    # for bn_stats.
                    outc = ep.tile([33, 1024], F32, tag="outc", name="outc")
                    nc.scalar.copy(outc, outX_ps)
                    last = (b == B - 1 and icb == NB - 1)
                    osl = bass.ds(icb * 1024, 1024)
                    if not last:
                        r128 = ep.tile([128, 8], F32, tag="r128", name="r128")
                        nc.gpsimd.dma_start(out=r128, in_=outc[32:33, :])
                        recip = ep.tile([128, 8], F32, tag="recip",
                                        name="recip")
                        nc.vector.reciprocal(recip, r128)
                        rdr = drp.tile([1, 1024], F32, tag="rdr", name="rdr")
                        nc.gpsimd.dma_start(out=rdr, in_=recip)
                        rbc = ep.tile([C, 1024], F32, tag="rbc", name="rbc")
                        nc.gpsimd.dma_start(out=rbc, in_=_bcast_rows(rdr, C))
                        nc.gpsimd.tensor_mul(outn_sb[b][:, osl],
                                             outc[0:32, :], rbc)
                    else:
                        # final block: shortest-latency path (direct bf16
                        # reciprocal, PE bcast, DVE mul) - nothing overlaps
                        # the tail
                        rr1 = ep.tile([1, 1024], BF16, tag="rr1", name="rr1")
                        with nc.allow_low_precision(
                                reason="rowsum recip feeds bf16 PE bcast"):
                            nc.vector.reciprocal(rr1, outc[32:33, :])
                    for hh in range(2):
                        hsl = bass.ds(icb * 1024 + hh * 512, 512)
                        if last:
                            rb_ps = pprojp.tile([C, 512], F32, tag="pp",
                                                name="rb_ps")
                            nc.tensor.matmul(rb_ps, lhsT=ones1_s,
                                             rhs=rr1[:, bass.ts(hh, 512)],
                                             start=True, stop=True)
                            nc.vector.tensor_mul(
                                outn_sb[b][:, hsl],
                                outc[0:32, bass.ts(hh, 512)], rb_ps)
                        y_ps = pprojp.tile([C, 512], F32, tag="pp",
                                           name="y_ps")
                        nc.tensor.matmul(y_ps, lhsT=w2_s,
                                         rhs=outn_sb[b][:, hsl],
                                         start=True, stop=True)
                        nc.vector.bn_stats(stats[:, blk * 2 + hh, :], y_ps)

                emit_proj(0)
                emit_block(0, 0, 0)
                emit_proj(1)
                emit_block(0, 1, 1)
                emit_block(0, 2, 2)
                for icb in range(NB):
                    emit_block(1, icb, NB + icb)

            # BatchNorm tail: aggregate stats, re-project, affine, store
            with tc.tile_pool(name="tail", bufs=1) as tail, \
                 tc.tile_pool(name="ps_tl", bufs=2, space="PSUM") as ps_tl:
                mv = tail.tile([C, 2], F32)
                nc.vector.bn_aggr(mv, stats)
                std = tail.tile([C, 1], F32)
                nc.scalar.activation(std, mv[:, 1:2], AFT.Sqrt,
                                     bias=epsb, scale=1.0)
                rstd = tail.tile([C, 1], F32)
                nc.vector.reciprocal(rstd, std)
                sc = tail.tile([C, 1], F32)
                nc.vector.tensor_mul(sc, gm_s, rstd)
                msc = tail.tile([C, 1], F32)
                nc.vector.tensor_mul(msc, mv[:, 0:1], sc)
                nb = tail.tile([C, 1], F32)
                nc.vector.tensor_sub(nb, bt_s, msc)
                k = 0
                for b in range(B):
                    for icx in range(NB):
                        sl = bass.ts(icx, 1024)
                        yt_ps = ps_tl.tile([C, 1024], F32, tag="yt",
                                           name="yt_ps")
                        for hh in range(2):
                            nc.tensor.matmul(
                                yt_ps[:, bass.ts(hh, 512)], lhsT=w2_s,
                                rhs=outn_sb[b][:, bass.ds(icx * 1024 +
                                                          hh * 512, 512)],
                                start=True, stop=True)
                        yo = tail.tile([C, 1024], F32, tag="yo", name="yo",
                                       bufs=4)
                        if k % 2 == 0:
                            nc.scalar.activation(yo, yt_ps, AFT.Identity,
                                                 bias=nb, scale=sc)
                        else:
                            nc.vector.tensor_scalar(
                                out=yo, in0=yt_ps, scalar1=sc, scalar2=nb,
                                op0=mybir.AluOpType.mult,
                                op1=mybir.AluOpType.add)
                        if k % 2 == 0:
                            nc.sync.dma_start(out=y_d[b][:, sl], in_=yo)
                        else:
                            nc.gpsimd.dma_start(out=y_d[b][:, sl], in_=yo)
                        k += 1

    nc.compile()
    return nc


_NC_CACHE = None


def _get_nc():
    global _NC_CACHE
    if _NC_CACHE is None:
        _NC_CACHE = build_program()
    return _NC_CACHE


def make_in_maps(x, wq, wk, wv, wo, gamma, beta):
    import ml_dtypes
    f = np.float32
    bf = ml_dtypes.bfloat16
    ones1 = np.ones((1, N), f).astype(bf)
    in_maps = []
    for h in range(H):
        cs = slice(h * C, (h + 1) * C)
        xh = np.ascontiguousarray(x[:, cs, :]).astype(bf)
        # x^T tiles with ones column: [B, 128, NT, 33]
        xt = np.ones((B, 128, NT, 33), f)
        xtr = np.ascontiguousarray(x[:, cs, :].transpose(0, 2, 1))  # [B,N,C]
        xt[:, :, :, :32] = xtr.reshape(B, NT, 128, C).transpose(0, 2, 1, 3)
        wall = np.empty((C, 96), f)
        wall[:, 0:32] = -2.0 * (wk[h].T @ wq[h])    # g-rows weight (G.T)
        wall[:, 32:64] = wq[h].T
        wall[:, 64:96] = wk[h].T
        in_maps.append({
            "xh": xh,
            "xt": xt.astype(bf),
            "on1": ones1,
            "wall": wall.astype(bf),
            "w2t": np.ascontiguousarray((wo[h] @ wv[h]).T.astype(f)),
            "gm": np.ascontiguousarray(gamma[cs].reshape(C, 1).astype(f)),
            "bt": np.ascontiguousarray(beta[cs].reshape(C, 1).astype(f)),
        })
    return in_maps


def kernel(x, wq, wk, wv, wo, bo, gamma, beta):
    x, wq, wk, wv, wo, gamma, beta = (np.asarray(a) for a in
                                      (x, wq, wk, wv, wo, gamma, beta))
    nc = _get_nc()
    in_maps = make_in_maps(x, wq, wk, wv, wo, gamma, beta)
    res = run_bass_kernel_spmd(nc, in_maps, list(range(H)))
    y = np.empty((B, DIM, N), np.float32)
    for h in range(H):
        y[:, h * C:(h + 1) * C, :] = res.results[h]["y"]
    return y


# revision 10
# speedup vs baseline: 1.3137x; 1.0019x over previous
"""Trainium2 Bass kernel for per-head L2-distance attention + grouped output
projection + BatchNorm (dense_transformer, B=2, dim=256, N=3072, H=8, D=32).

Sharding: one head per NeuronCore (8 heads = 8 cores), both batches on each
core.  Channels split by head, so the BatchNorm per-channel (b, n) reduction
is fully core-local -> zero collectives.

v3 design (vs v2 @ 225.8us):
  - Same folded math as v2: W = -2 wq^T wk contracts x against g = W^T x;
    augmented K=96 contraction gives ST[j,i] = ||q_i - k_j||^2 in one matmul
    pair; W2 = (wo wv)^T collapses PV + output projection; exp chain is
    ACT Sqrt (t-domain) -> custom DVE cubic+3sq poly (precision-critical:
    BN divides by tiny per-channel variance, ~50x error amplification).
  - DVE exp ops are TRIPLE-wide ([128, 3072] spanning 3 j-tiles): DVE cost
    is free-size-driven, so 48 ops x ~3.3us beat 144 x ~1.2us by ~15us.
    The DVE was the saturated engine (98% busy in-loop).
  - PSUM re-quadrant: one persistent [128, 1024] accumulator tile holds
    outX at partitions 64:97 (PV accumulation, tile_position col 64),
    y = W2@outn at partitions 0:32 (bn_stats source), and the last-block
    rowsum-broadcast at 32:64.  That frees 2 banks -> ps_st bufs=3, so the
    ACT never waits on ST matmul double-buffering (block-boundary stalls
    were ~26us).  Engines handle mismatched in/out base partitions for
    built-in ops (probed on HW); only custom DVE ops are base-0-only.
  - outX -> SBUF epilogue copy moved ACT -> DVE (fits the DVE's block-
    boundary slack; keeps ACT streaming sqrts across block boundaries).
  - ones rows of qa/ka come from a [1, N] DRAM row via partition-stride-0
    broadcast DMA (was 786KB of host-staged ones).
  - b=1 prologue projections are emitted after b=0's first main block so
    their PSUM slots don't gate b=0 loop start.
  - Tail: last block skips the [128,8] reshape (direct [1,1024] bf16
    reciprocal feeds the PE rowsum broadcast); affines alternate ACT/DVE
    3/3; output DMAs alternate queues.
  Known dead ends (measured): fp8-DoubleRow ST (2.2e-2 rel err) and fp8 PV
  (5.7e-2 sim), u-domain DVE-only exp poly (systematic 1.3% -> 7e-2 after
  BN), >512-free matmul (fp32 PSUM out is mandatory on TRN2), bf16 PSUM
  (TRN3-only), DMA cannot touch PSUM, custom DVE 2x perf mode (slice
  budget 2x7>8), exp-on-ACT rebalance (Sqrt and Exp live in different
  1.28us-swap ACT tables), GPSIMD exp (no library; toolchain unavailable),
  bn_stats >512 free (HW limit).
"""

import numpy as np

import concourse.bass as bass
import concourse.tile as tile
from concourse import bacc, mybir
from concourse.bass_utils import run_bass_kernel_spmd

F32 = mybir.dt.float32
F32R = mybir.dt.float32r
BF16 = mybir.dt.bfloat16
AFT = mybir.ActivationFunctionType

B, DIM, N, H, D = 2, 256, 3072, 8, 32
C = DIM // H          # 32 input channels per head
NT = N // 128         # 24 j-tiles
NB = N // 1024        # 3 i-blocks per batch
EPS_BN = 1e-5

# --- exp polynomial calibration (baseline-proven, rel err ~1.5e-6) ---
# w = sqrt(d2)/(8*sqrt(32)); minimax cubic for exp(-t/gam) on t in [0, W*gam],
# rescaled so the cubic coefficient is -1:  p(t) = B0 + t*(B1 + t*(B2 - t)),
# out = ((p^2)^2)^2 = exp(-sqrt(d2)/sqrt(32)).
B0 = 0.999999894052468
B1 = -1.858707805584652
B2 = 1.7242982194980068
ACT_SCALE = 0.00014132731  # (gam*scale/8)^2
ACT_BIAS = 1.413273e-09    # ACT_SCALE * 1e-5 protective epsilon inside sqrt

_EXP_OP = None


def _register_exp_op():
    """Register the exp(-.) polynomial as a custom DVE op (in-process)."""
    global _EXP_OP
    if _EXP_OP is not None:
        return _EXP_OP
    import concourse.dve_ops as dve_ops
    from concourse.dve_spec import Spec, Src0, C0, C1, C2, sq, lower, _has_src1
    from concourse.dve_uop import DveOpSpec

    name = "EXP_NEG_POLY3SQ3_ANT"
    for o in dve_ops.OPS:
        if o.name == name:
            _EXP_OP = o
            return o

    t = Src0
    body = sq(sq(sq(C0 + t * (C1 + t * (C2 - t)))))

    def ref(in0, in1, c0, c1, c2):
        tt = in0.astype(np.float32)
        p = (c0 + tt * (c1 + tt * (c2 - tt))).astype(np.float32)
        for _ in range(3):
            p = (p * p).astype(np.float32)
        return p

    spec = Spec(body=body, reference=ref)
    row = dve_ops._CUSTOM_DVE_ROW_BASE + len(dve_ops.OPS)
    shas = {}
    for ver in ("v3", "v4"):
        try:
            uops = lower(spec, ver=ver)
            s = DveOpSpec(name=name, opcode=row, uops=uops, rd1_en=_has_src1(spec))
            shas[ver] = s.sha(ver)
        except Exception:
            pass
    op = dve_ops.DveOp(name, spec, subdim=False, uops_sha=shas)
    dve_ops.OPS.append(op)
    dve_ops._SUB_OPCODE_FOR_NAME[name] = row
    dve_ops.CUSTOM_DVE_SPECS[name] = spec
    _EXP_OP = op
    return op


def _bcast_rows(ap: bass.AP, nrows: int) -> bass.AP:
    """[1, n] AP -> partition-stride-0 [nrows, n] AP (for DMA replicate)."""
    return bass.AP(tensor=ap.tensor, offset=ap.offset, ap=[[0, nrows], ap.ap[-1]])


def build_program():
    exp_op = _register_exp_op()
    nc = bacc.Bacc("TRN2", target_bir_lowering=False, debug=False)

    xh_d = nc.dram_tensor("xh", [B, C, N], BF16, kind="ExternalInput").ap()
    xt_d = nc.dram_tensor("xt", [B, 128, NT, 33], BF16, kind="ExternalInput").ap()
    on_d = nc.dram_tensor("on1", [1, N], BF16, kind="ExternalInput").ap()
    wall_d = nc.dram_tensor("wall", [C, 96], BF16, kind="ExternalInput").ap()
    w2_d = nc.dram_tensor("w2t", [C, C], F32R, kind="ExternalInput").ap()
    gm_d = nc.dram_tensor("gm", [C, 1], F32, kind="ExternalInput").ap()
    bt_d = nc.dram_tensor("bt", [C, 1], F32, kind="ExternalInput").ap()
    y_d = nc.dram_tensor("y", [B, C, N], F32, kind="ExternalOutput").ap()

    with tile.TileContext(nc) as tc:
        with tc.tile_pool(name="const", bufs=1) as const, \
             tc.tile_pool(name="persist", bufs=1) as persist, \
             tc.tile_pool(name="drp", bufs=2, space="DRAM") as drp:
            wall_s = const.tile([C, 96], BF16)
            w2_s = const.tile([C, C], F32R)
            gm_s = const.tile([C, 1], F32)
            bt_s = const.tile([C, 1], F32)
            actb = const.tile([128, 1], F32)
            epsb = const.tile([C, 1], F32)
            ones1_s = const.tile([1, C], BF16)
            nc.vector.memset(ones1_s, 1.0)
            for dst, src in ((wall_s, wall_d), (w2_s, w2_d),
                             (gm_s, gm_d), (bt_s, bt_d)):
                nc.sync.dma_start(out=dst, in_=src)
            nc.vector.memset(actb, ACT_BIAS)
            nc.vector.memset(epsb, EPS_BN)
            # force the (single) sqrt_and_others table load off the critical
            # path: first ACT op is a Sqrt, so Square ops later reuse the set
            warm = const.tile([1, 1], F32)
            nc.scalar.activation(warm, epsb[0:1, :], AFT.Sqrt,
                                 bias=0.0, scale=1.0)

            outn_sb = [persist.tile([C, N], F32R, tag=f"on{b}", name=f"outn{b}")
                       for b in range(B)]
            stats = persist.tile([C, B * NB * 2, 6], F32)

            with tc.tile_pool(name="xb", bufs=1) as xbp, \
                 tc.tile_pool(name="pproj", bufs=1, space="PSUM") as pprojp, \
                 tc.tile_pool(name="st", bufs=2, space="PSUM") as ps_st, \
                 tc.tile_pool(name="acc", bufs=1, space="PSUM") as ps_acc, \
                 tc.tile_pool(name="mt", bufs=2) as mt, \
                 tc.tile_pool(name="ep", bufs=2) as ep:
                pro = {}
                for b in range(B):
                    qa = xbp.tile([96, N], BF16, tag=f"qa{b}", name=f"qa{b}")
                    ka = xbp.tile([96, N], BF16, tag=f"ka{b}", name=f"ka{b}")
                    xaT = xbp.tile([128, NT, 33], BF16, tag=f"xaT{b}",
                                   name=f"xaT{b}")
                    # x straight into qa's qk-rows; ones rows broadcast from
                    # a [1, N] DRAM row; x^T (with ones col) from host
                    nc.gpsimd.dma_start(out=qa[0:32, 0:1024],
                                        in_=xh_d[b][:, 0:1024])
                    nc.gpsimd.dma_start(out=qa[0:32, 1024:3072],
                                        in_=xh_d[b][:, 1024:3072])
                    nc.sync.dma_start(out=qa[64:96, :], in_=_bcast_rows(on_d, 32))
                    nc.sync.dma_start(out=ka[32:64, :], in_=_bcast_rows(on_d, 32))
                    nc.gpsimd.dma_start(out=xaT, in_=xt_d[b])
                    pro[b] = (qa, ka, xaT)

                def emit_proj(b):
                    # combined projection: rows 0-31 g = W^T x, 32-63 q,
                    # 64-95 k.  g-copy on DVE (custom-op-free, base 0),
                    # squares on ACT.
                    qa, ka, _ = pro[b]
                    for icx in range(NB):
                        sl = bass.ts(icx, 1024)
                        pa = pprojp.tile([96, 1024], F32, tag="pp",
                                         name="pa")
                        for hh in range(2):
                            nc.tensor.matmul(
                                pa[:, bass.ts(hh, 512)], lhsT=wall_s,
                                rhs=qa[0:32, bass.ds(icx * 1024 + hh * 512,
                                                     512)],
                                start=True, stop=True)
                        nc.vector.tensor_copy(ka[0:32, sl], pa[0:32, :])
                        nc.scalar.activation(qa[32:64, sl], pa[32:64, :],
                                             AFT.Square, bias=0.0, scale=1.0)
                        nc.scalar.activation(ka[64:96, sl], pa[64:96, :],
                                             AFT.Square, bias=0.0, scale=1.0)

                def emit_block(b, icb, blk):
                    qa, ka, xaT = pro[b]
                    outX_ps = ps_acc.tile([33, 1024], F32, tag="outX",
                                          name="outX_ps")
                    for jt in range(NT):
                        st_t = ps_st.tile([128, 1024], F32, tag="st",
                                          name="st_ps")
                        for hh in range(2):
                            nsl = bass.ds(icb * 1024 + hh * 512, 512)
                            nc.tensor.matmul(
                                st_t[:, bass.ts(hh, 512)],
                                lhsT=ka[:, bass.ts(jt, 128)],
                                rhs=qa[:, nsl],
                                start=True, stop=True)
                        t_sb = mt.tile([128, 1024], F32, tag="t",
                                       name="t_sb", bufs=4)
                        nc.scalar.activation(t_sb, st_t, AFT.Sqrt,
                                             bias=actb, scale=ACT_SCALE)
                        p_sb = mt.tile([128, 1024], BF16, tag="p",
                                       name="p_sb", bufs=4)
                        nc.vector._custom_dve(exp_op, out=p_sb, in0=t_sb,
                                              s0=B0, s1=B1, imm2=B2)
                        for hh in range(2):
                            nc.tensor.matmul(
                                outX_ps[:, bass.ts(hh, 512)],
                                lhsT=xaT[:, jt, :],
                                rhs=p_sb[:, bass.ts(hh, 512)],
                                start=(jt == 0),
                                stop=(jt == NT - 1))
                    # epilogue: one ACT pass brings outX (incl rowsum row)
                    # to SBUF, then rowsum recip, normalize, y = W2@outn
                    # for bn_stats.
                    outc = ep.tile([33, 1024], F32, tag="outc", name="outc")
                    nc.scalar.copy(outc, outX_ps)
                    last = (b == B - 1 and icb == NB - 1)
                    osl = bass.ds(icb * 1024, 1024)
                    if not last:
                        r128 = ep.tile([128, 8], F32, tag="r128", name="r128")
                        nc.gpsimd.dma_start(out=r128, in_=outc[32:33, :])
                        recip = ep.tile([128, 8], F32, tag="recip",
                                        name="recip")
                        nc.vector.reciprocal(recip, r128)
                        rdr = drp.tile([1, 1024], F32, tag="rdr", name="rdr")
                        nc.gpsimd.dma_start(out=rdr, in_=recip)
                        rbc = ep.tile([C, 1024], F32, tag="rbc", name="rbc")
                        nc.gpsimd.dma_start(out=rbc, in_=_bcast_rows(rdr, C))
                        nc.gpsimd.tensor_mul(outn_sb[b][:, osl],
                                             outc[0:32, :], rbc)
                    else:
                        # final block: lowest-latency path (PE bcast + DVE
                        # mul; [128,8] reshape because DVE reciprocal costs
                        # ~6 cyc/elem, so [1,1024] direct would be 6.5us)
                        r128 = ep.tile([128, 8], F32, tag="r128", name="r128")
                        nc.gpsimd.dma_start(out=r128, in_=outc[32:33, :])
                        recip = ep.tile([128, 8], F32, tag="recip",
                                        name="recip")
                        nc.vector.reciprocal(recip, r128)
                        rr1 = ep.tile([1, 1024], BF16, tag="rr1", name="rr1")
                        nc.gpsimd.dma_start(out=rr1, in_=recip)
                    for hh in range(2):
                        hsl = bass.ds(icb * 1024 + hh * 512, 512)
                        if last:
                            rb_ps = pprojp.tile([C, 512], F32, tag="pp",
                                                name="rb_ps")
                            nc.tensor.matmul(rb_ps, lhsT=ones1_s,
                                             rhs=rr1[:, bass.ts(hh, 512)],
                                             start=True, stop=True)
                            nc.vector.tensor_mul(
                                outn_sb[b][:, hsl],
                                outc[0:32, bass.ts(hh, 512)], rb_ps)
                        y_ps = pprojp.tile([C, 512], F32, tag="pp",
                                           name="y_ps")
                        nc.tensor.matmul(y_ps, lhsT=w2_s,
                                         rhs=outn_sb[b][:, hsl],
                                         start=True, stop=True)
                        nc.vector.bn_stats(stats[:, blk * 2 + hh, :], y_ps)

                emit_proj(0)
                emit_block(0, 0, 0)
                emit_proj(1)
                emit_block(0, 1, 1)
                emit_block(0, 2, 2)
                for icb in range(NB):
                    emit_block(1, icb, NB + icb)

            # BatchNorm tail: aggregate stats, re-project, affine, store.
            # The W2 re-projections depend only on outn, not on the stats,
            # so all six are issued first (PSUM-parked, bufs=4) and run
            # during the last-block epilogue + aggregation; each affine is
            # split across ACT (low half) and DVE (high half).
            with tc.tile_pool(name="tail", bufs=1) as tail, \
                 tc.tile_pool(name="ps_tl", bufs=4, space="PSUM") as ps_tl:
                yts = []
                for b in range(B):
                    for icx in range(NB):
                        yt_ps = ps_tl.tile([C, 1024], F32, tag="yt",
                                           name="yt_ps")
                        for hh in range(2):
                            nc.tensor.matmul(
                                yt_ps[:, bass.ts(hh, 512)], lhsT=w2_s,
                                rhs=outn_sb[b][:, bass.ds(icx * 1024 +
                                                          hh * 512, 512)],
                                start=True, stop=True)
                        yts.append((b, icx, yt_ps))
                mv = tail.tile([C, 2], F32)
                nc.vector.bn_aggr(mv, stats)
                std = tail.tile([C, 1], F32)
                nc.scalar.activation(std, mv[:, 1:2], AFT.Sqrt,
                                     bias=epsb, scale=1.0)
                rstd = tail.tile([C, 1], F32)
                nc.vector.reciprocal(rstd, std)
                sc = tail.tile([C, 1], F32)
                nc.vector.tensor_mul(sc, gm_s, rstd)
                msc = tail.tile([C, 1], F32)
                nc.vector.tensor_mul(msc, mv[:, 0:1], sc)
                nb = tail.tile([C, 1], F32)
                nc.vector.tensor_sub(nb, bt_s, msc)
                for k, (b, icx, yt_ps) in enumerate(yts):
                    sl = bass.ts(icx, 1024)
                    yo = tail.tile([C, 1024], F32, tag="yo", name="yo",
                                   bufs=4)
                    nc.scalar.activation(yo[:, 0:512], yt_ps[:, 0:512],
                                         AFT.Identity, bias=nb, scale=sc)
                    nc.vector.tensor_scalar(
                        out=yo[:, 512:1024], in0=yt_ps[:, 512:1024],
                        scalar1=sc, scalar2=nb,
                        op0=mybir.AluOpType.mult,
                        op1=mybir.AluOpType.add)
                    if k % 2 == 0:
                        nc.sync.dma_start(out=y_d[b][:, sl], in_=yo)
                    else:
                        nc.gpsimd.dma_start(out=y_d[b][:, sl], in_=yo)

    nc.compile()
    return nc


_NC_CACHE = None


def _get_nc():
    global _NC_CACHE
    if _NC_CACHE is None:
        _NC_CACHE = build_program()
    return _NC_CACHE


def make_in_maps(x, wq, wk, wv, wo, gamma, beta):
    import ml_dtypes
    f = np.float32
    bf = ml_dtypes.bfloat16
    ones1 = np.ones((1, N), f).astype(bf)
    in_maps = []
    for h in range(H):
        cs = slice(h * C, (h + 1) * C)
        xh = np.ascontiguousarray(x[:, cs, :]).astype(bf)
        # x^T tiles with ones column: [B, 128, NT, 33]
        xt = np.ones((B, 128, NT, 33), f)
        xtr = np.ascontiguousarray(x[:, cs, :].transpose(0, 2, 1))  # [B,N,C]
        xt[:, :, :, :32] = xtr.reshape(B, NT, 128, C).transpose(0, 2, 1, 3)
        wall = np.empty((C, 96), f)
        wall[:, 0:32] = -2.0 * (wk[h].T @ wq[h])    # g-rows weight (G.T)
        wall[:, 32:64] = wq[h].T
        wall[:, 64:96] = wk[h].T
        in_maps.append({
            "xh": xh,
            "xt": xt.astype(bf),
            "on1": ones1,
            "wall": wall.astype(bf),
            "w2t": np.ascontiguousarray((wo[h] @ wv[h]).T.astype(f)),
            "gm": np.ascontiguousarray(gamma[cs].reshape(C, 1).astype(f)),
            "bt": np.ascontiguousarray(beta[cs].reshape(C, 1).astype(f)),
        })
    return in_maps


def kernel(x, wq, wk, wv, wo, bo, gamma, beta):
    x, wq, wk, wv, wo, gamma, beta = (np.asarray(a) for a in
                                      (x, wq, wk, wv, wo, gamma, beta))
    nc = _get_nc()
    in_maps = make_in_maps(x, wq, wk, wv, wo, gamma, beta)
    res = run_bass_kernel_spmd(nc, in_maps, list(range(H)))
    y = np.empty((B, DIM, N), np.float32)
    for h in range(H):
        y[:, h * C:(h + 1) * C, :] = res.results[h]["y"]
    return y


# revision 13
# speedup vs baseline: 1.3311x; 1.0132x over previous
"""Trainium2 Bass kernel for per-head L2-distance attention + grouped output
projection + BatchNorm (dense_transformer, B=2, dim=256, N=3072, H=8, D=32).

Sharding: one head per NeuronCore (8 heads = 8 cores), both batches on each
core.  Channels split by head, so the BatchNorm per-channel (b, n) reduction
is fully core-local -> zero collectives.

v3 design (vs v2 @ 225.8us):
  - Same folded math as v2: W = -2 wq^T wk contracts x against g = W^T x;
    augmented K=96 contraction gives ST[j,i] = ||q_i - k_j||^2 in one matmul
    pair; W2 = (wo wv)^T collapses PV + output projection; exp chain is
    ACT Sqrt (t-domain) -> custom DVE cubic+3sq poly (precision-critical:
    BN divides by tiny per-channel variance, ~50x error amplification).
  - DVE exp ops are TRIPLE-wide ([128, 3072] spanning 3 j-tiles): DVE cost
    is free-size-driven, so 48 ops x ~3.3us beat 144 x ~1.2us by ~15us.
    The DVE was the saturated engine (98% busy in-loop).
  - PSUM re-quadrant: one persistent [128, 1024] accumulator tile holds
    outX at partitions 64:97 (PV accumulation, tile_position col 64),
    y = W2@outn at partitions 0:32 (bn_stats source), and the last-block
    rowsum-broadcast at 32:64.  That frees 2 banks -> ps_st bufs=3, so the
    ACT never waits on ST matmul double-buffering (block-boundary stalls
    were ~26us).  Engines handle mismatched in/out base partitions for
    built-in ops (probed on HW); only custom DVE ops are base-0-only.
  - outX -> SBUF epilogue copy moved ACT -> DVE (fits the DVE's block-
    boundary slack; keeps ACT streaming sqrts across block boundaries).
  - ones rows of qa/ka come from a [1, N] DRAM row via partition-stride-0
    broadcast DMA (was 786KB of host-staged ones).
  - b=1 prologue projections are emitted after b=0's first main block so
    their PSUM slots don't gate b=0 loop start.
  - Tail: last block skips the [128,8] reshape (direct [1,1024] bf16
    reciprocal feeds the PE rowsum broadcast); affines alternate ACT/DVE
    3/3; output DMAs alternate queues.
  Known dead ends (measured): fp8-DoubleRow ST (2.2e-2 rel err) and fp8 PV
  (5.7e-2 sim), u-domain DVE-only exp poly (systematic 1.3% -> 7e-2 after
  BN), >512-free matmul (fp32 PSUM out is mandatory on TRN2), bf16 PSUM
  (TRN3-only), DMA cannot touch PSUM, custom DVE 2x perf mode (slice
  budget 2x7>8), exp-on-ACT rebalance (Sqrt and Exp live in different
  1.28us-swap ACT tables), GPSIMD exp (no library; toolchain unavailable),
  bn_stats >512 free (HW limit).
"""

import numpy as np

import concourse.bass as bass
import concourse.tile as tile
from concourse import bacc, mybir
from concourse.bass_utils import run_bass_kernel_spmd

F32 = mybir.dt.float32
F32R = mybir.dt.float32r
BF16 = mybir.dt.bfloat16
AFT = mybir.ActivationFunctionType

B, DIM, N, H, D = 2, 256, 3072, 8, 32
C = DIM // H          # 32 input channels per head
NT = N // 128         # 24 j-tiles
NB = N // 1024        # 3 i-blocks per batch
EPS_BN = 1e-5

# --- exp polynomial calibration (baseline-proven, rel err ~1.5e-6) ---
# w = sqrt(d2)/(8*sqrt(32)); minimax cubic for exp(-t/gam) on t in [0, W*gam],
# rescaled so the cubic coefficient is -1:  p(t) = B0 + t*(B1 + t*(B2 - t)),
# out = ((p^2)^2)^2 = exp(-sqrt(d2)/sqrt(32)).
B0 = 0.999999894052468
B1 = -1.858707805584652
B2 = 1.7242982194980068
ACT_SCALE = 0.00014132731  # (gam*scale/8)^2
ACT_BIAS = 1.413273e-09    # ACT_SCALE * 1e-5 protective epsilon inside sqrt

_EXP_OP = None


def _register_exp_op():
    """Register the exp(-.) polynomial as a custom DVE op (in-process)."""
    global _EXP_OP
    if _EXP_OP is not None:
        return _EXP_OP
    import concourse.dve_ops as dve_ops
    from concourse.dve_spec import Spec, Src0, C0, C1, C2, sq, lower, _has_src1
    from concourse.dve_uop import DveOpSpec

    name = "EXP_NEG_POLY3SQ3_ANT"
    for o in dve_ops.OPS:
        if o.name == name:
            _EXP_OP = o
            return o

    t = Src0
    body = sq(sq(sq(C0 + t * (C1 + t * (C2 - t)))))

    def ref(in0, in1, c0, c1, c2):
        tt = in0.astype(np.float32)
        p = (c0 + tt * (c1 + tt * (c2 - tt))).astype(np.float32)
        for _ in range(3):
            p = (p * p).astype(np.float32)
        return p

    spec = Spec(body=body, reference=ref)
    row = dve_ops._CUSTOM_DVE_ROW_BASE + len(dve_ops.OPS)
    shas = {}
    for ver in ("v3", "v4"):
        try:
            uops = lower(spec, ver=ver)
            s = DveOpSpec(name=name, opcode=row, uops=uops, rd1_en=_has_src1(spec))
            shas[ver] = s.sha(ver)
        except Exception:
            pass
    op = dve_ops.DveOp(name, spec, subdim=False, uops_sha=shas)
    dve_ops.OPS.append(op)
    dve_ops._SUB_OPCODE_FOR_NAME[name] = row
    dve_ops.CUSTOM_DVE_SPECS[name] = spec
    _EXP_OP = op
    return op


def _bcast_rows(ap: bass.AP, nrows: int) -> bass.AP:
    """[1, n] AP -> partition-stride-0 [nrows, n] AP (for DMA replicate)."""
    return bass.AP(tensor=ap.tensor, offset=ap.offset, ap=[[0, nrows], ap.ap[-1]])


def build_program():
    exp_op = _register_exp_op()
    nc = bacc.Bacc("TRN2", target_bir_lowering=False, debug=False)

    xh_d = nc.dram_tensor("xh", [B, C, N], BF16, kind="ExternalInput").ap()
    xt_d = nc.dram_tensor("xt", [B, 128, NT, 33], BF16, kind="ExternalInput").ap()
    on_d = nc.dram_tensor("on1", [1, N], BF16, kind="ExternalInput").ap()
    wall_d = nc.dram_tensor("wall", [C, 96], BF16, kind="ExternalInput").ap()
    w2_d = nc.dram_tensor("w2t", [C, C], F32R, kind="ExternalInput").ap()
    gm_d = nc.dram_tensor("gm", [C, 1], F32, kind="ExternalInput").ap()
    bt_d = nc.dram_tensor("bt", [C, 1], F32, kind="ExternalInput").ap()
    y_d = nc.dram_tensor("y", [B, C, N], F32, kind="ExternalOutput").ap()

    with tile.TileContext(nc) as tc:
        with tc.tile_pool(name="const", bufs=1) as const, \
             tc.tile_pool(name="persist", bufs=1) as persist, \
             tc.tile_pool(name="drp", bufs=2, space="DRAM") as drp:
            wall_s = const.tile([C, 96], BF16)
            w2_s = const.tile([C, C], F32R)
            gm_s = const.tile([C, 1], F32)
            bt_s = const.tile([C, 1], F32)
            actb = const.tile([128, 1], F32)
            epsb = const.tile([C, 1], F32)
            ones1_s = const.tile([1, C], BF16)
            nc.vector.memset(ones1_s, 1.0)
            for dst, src in ((wall_s, wall_d), (w2_s, w2_d),
                             (gm_s, gm_d), (bt_s, bt_d)):
                nc.sync.dma_start(out=dst, in_=src)
            nc.vector.memset(actb, ACT_BIAS)
            nc.vector.memset(epsb, EPS_BN)
            # force the (single) sqrt_and_others table load off the critical
            # path: first ACT op is a Sqrt, so Square ops later reuse the set
            warm = const.tile([1, 1], F32)
            nc.scalar.activation(warm, epsb[0:1, :], AFT.Sqrt,
                                 bias=0.0, scale=1.0)

            outn_sb = [persist.tile([C, N], F32R, tag=f"on{b}", name=f"outn{b}")
                       for b in range(B)]
            stats = persist.tile([C, B * NB * 2, 6], F32)

            with tc.tile_pool(name="xb", bufs=1) as xbp, \
                 tc.tile_pool(name="pproj", bufs=1, space="PSUM") as pprojp, \
                 tc.tile_pool(name="st", bufs=2, space="PSUM") as ps_st, \
                 tc.tile_pool(name="acc", bufs=1, space="PSUM") as ps_acc, \
                 tc.tile_pool(name="mt", bufs=2) as mt, \
                 tc.tile_pool(name="ep", bufs=2) as ep:
                pro = {}
                for b in range(B):
                    qa = xbp.tile([96, N], BF16, tag=f"qa{b}", name=f"qa{b}")
                    ka = xbp.tile([96, N], BF16, tag=f"ka{b}", name=f"ka{b}")
                    xaT = xbp.tile([128, NT, 33], BF16, tag=f"xaT{b}",
                                   name=f"xaT{b}")
                    # x straight into qa's qk-rows; ones rows broadcast from
                    # a [1, N] DRAM row; x^T (with ones col) from host
                    nc.gpsimd.dma_start(out=qa[0:32, 0:1024],
                                        in_=xh_d[b][:, 0:1024])
                    nc.gpsimd.dma_start(out=qa[0:32, 1024:3072],
                                        in_=xh_d[b][:, 1024:3072])
                    nc.sync.dma_start(out=qa[64:96, :], in_=_bcast_rows(on_d, 32))
                    nc.sync.dma_start(out=ka[32:64, :], in_=_bcast_rows(on_d, 32))
                    nc.gpsimd.dma_start(out=xaT, in_=xt_d[b])
                    pro[b] = (qa, ka, xaT)

                def emit_proj(b):
                    # combined projection: rows 0-31 g = W^T x, 32-63 q,
                    # 64-95 k.  g-copy on DVE (custom-op-free, base 0),
                    # squares on ACT.
                    qa, ka, _ = pro[b]
                    for icx in range(NB):
                        sl = bass.ts(icx, 1024)
                        pa = pprojp.tile([96, 1024], F32, tag="pp",
                                         name="pa")
                        for hh in range(2):
                            nc.tensor.matmul(
                                pa[:, bass.ts(hh, 512)], lhsT=wall_s,
                                rhs=qa[0:32, bass.ds(icx * 1024 + hh * 512,
                                                     512)],
                                start=True, stop=True)
                        nc.vector.tensor_copy(ka[0:32, sl], pa[0:32, :])
                        nc.scalar.activation(qa[32:64, sl], pa[32:64, :],
                                             AFT.Square, bias=0.0, scale=1.0)
                        nc.scalar.activation(ka[64:96, sl], pa[64:96, :],
                                             AFT.Square, bias=0.0, scale=1.0)

                def emit_block(b, icb, blk):
                    qa, ka, xaT = pro[b]
                    outX_ps = ps_acc.tile([33, 1024], F32, tag="outX",
                                          name="outX_ps")
                    for jt in range(NT):
                        st_t = ps_st.tile([128, 1024], F32, tag="st",
                                          name="st_ps")
                        for hh in range(2):
                            nsl = bass.ds(icb * 1024 + hh * 512, 512)
                            nc.tensor.matmul(
                                st_t[:, bass.ts(hh, 512)],
                                lhsT=ka[:, bass.ts(jt, 128)],
                                rhs=qa[:, nsl],
                                start=True, stop=True)
                        t_sb = mt.tile([128, 1024], F32, tag="t",
                                       name="t_sb", bufs=4)
                        nc.scalar.activation(t_sb, st_t, AFT.Sqrt,
                                             bias=actb, scale=ACT_SCALE)
                        p_sb = mt.tile([128, 1024], BF16, tag="p",
                                       name="p_sb", bufs=4)
                        nc.vector._custom_dve(exp_op, out=p_sb, in0=t_sb,
                                              s0=B0, s1=B1, imm2=B2)
                        for hh in range(2):
                            nc.tensor.matmul(
                                outX_ps[:, bass.ts(hh, 512)],
                                lhsT=xaT[:, jt, :],
                                rhs=p_sb[:, bass.ts(hh, 512)],
                                start=(jt == 0),
                                stop=(jt == NT - 1))
                    # epilogue: one ACT pass brings outX (incl rowsum row)
                    # to SBUF, then rowsum recip, normalize, y = W2@outn
                    # for bn_stats.
                    outc = ep.tile([33, 1024], F32, tag="outc", name="outc")
                    nc.scalar.copy(outc, outX_ps)
                    last = (b == B - 1 and icb == NB - 1)
                    osl = bass.ds(icb * 1024, 1024)
                    if not last:
                        r128 = ep.tile([128, 8], F32, tag="r128", name="r128")
                        nc.gpsimd.dma_start(out=r128, in_=outc[32:33, :])
                        recip = ep.tile([128, 8], F32, tag="recip",
                                        name="recip")
                        nc.vector.reciprocal(recip, r128)
                        rdr = drp.tile([1, 1024], F32, tag="rdr", name="rdr")
                        nc.gpsimd.dma_start(out=rdr, in_=recip)
                        rbc = ep.tile([C, 1024], F32, tag="rbc", name="rbc")
                        nc.gpsimd.dma_start(out=rbc, in_=_bcast_rows(rdr, C))
                        nc.gpsimd.tensor_mul(outn_sb[b][:, osl],
                                             outc[0:32, :], rbc)
                    else:
                        # final block: lowest-latency path (PE bcast + DVE
                        # mul; [128,8] reshape because DVE reciprocal costs
                        # ~6 cyc/elem, so [1,1024] direct would be 6.5us)
                        r128 = ep.tile([128, 8], F32, tag="r128", name="r128")
                        nc.gpsimd.dma_start(out=r128, in_=outc[32:33, :])
                        recip = ep.tile([128, 8], F32, tag="recip",
                                        name="recip")
                        nc.vector.reciprocal(recip, r128)
                        rr1 = ep.tile([1, 1024], BF16, tag="rr1", name="rr1")
                        nc.gpsimd.dma_start(out=rr1, in_=recip)
                    if last:
                        # widen the final epilogue: both hh rowsum
                        # broadcasts first, then both muls - the hh chains
                        # overlap instead of serializing.  rb/y use [C,1024]
                        # tiles with disjoint hh halves (same "pp" slot
                        # footprint, no false buffer-rotation deps).
                        rb2 = pprojp.tile([C, 1024], F32, tag="pp",
                                          name="rb2")
                        for hh in range(2):
                            nc.tensor.matmul(rb2[:, bass.ts(hh, 512)],
                                             lhsT=ones1_s,
                                             rhs=rr1[:, bass.ts(hh, 512)],
                                             start=True, stop=True)
                        for hh in range(2):
                            hsl = bass.ds(icb * 1024 + hh * 512, 512)
                            nc.vector.tensor_mul(
                                outn_sb[b][:, hsl],
                                outc[0:32, bass.ts(hh, 512)],
                                rb2[:, bass.ts(hh, 512)])
                    y2 = pprojp.tile([C, 1024], F32, tag="pp", name="y2")
                    for hh in range(2):
                        hsl = bass.ds(icb * 1024 + hh * 512, 512)
                        nc.tensor.matmul(y2[:, bass.ts(hh, 512)],
                                         lhsT=w2_s,
                                         rhs=outn_sb[b][:, hsl],
                                         start=True, stop=True)
                        nc.vector.bn_stats(stats[:, blk * 2 + hh, :],
                                           y2[:, bass.ts(hh, 512)])

                emit_proj(0)
                emit_block(0, 0, 0)
                emit_proj(1)
                emit_block(0, 1, 1)
                emit_block(0, 2, 2)
                for icb in range(NB):
                    emit_block(1, icb, NB + icb)

            # BatchNorm tail: aggregate stats, re-project, affine, store.
            # The W2 re-projections depend only on outn, not on the stats,
            # so all six are issued first (PSUM-parked, bufs=4) and run
            # during the last-block epilogue + aggregation; each affine is
            # split across ACT (low half) and DVE (high half).
            with tc.tile_pool(name="tail", bufs=1) as tail, \
                 tc.tile_pool(name="ps_tl", bufs=4, space="PSUM") as ps_tl:
                yts = []
                for b in range(B):
                    for icx in range(NB):
                        yt_ps = ps_tl.tile([C, 1024], F32, tag="yt",
                                           name="yt_ps")
                        for hh in range(2):
                            nc.tensor.matmul(
                                yt_ps[:, bass.ts(hh, 512)], lhsT=w2_s,
                                rhs=outn_sb[b][:, bass.ds(icx * 1024 +
                                                          hh * 512, 512)],
                                start=True, stop=True)
                        yts.append((b, icx, yt_ps))
                mv = tail.tile([C, 2], F32)
                nc.vector.bn_aggr(mv, stats)
                std = tail.tile([C, 1], F32)
                nc.scalar.activation(std, mv[:, 1:2], AFT.Sqrt,
                                     bias=epsb, scale=1.0)
                rstd = tail.tile([C, 1], F32)
                nc.vector.reciprocal(rstd, std)
                sc = tail.tile([C, 1], F32)
                nc.vector.tensor_mul(sc, gm_s, rstd)
                msc = tail.tile([C, 1], F32)
                nc.vector.tensor_mul(msc, mv[:, 0:1], sc)
                nb = tail.tile([C, 1], F32)
                nc.vector.tensor_sub(nb, bt_s, msc)
                for k, (b, icx, yt_ps) in enumerate(yts):
                    sl = bass.ts(icx, 1024)
                    yo = tail.tile([C, 1024], F32, tag="yo", name="yo",
                                   bufs=6)
                    nc.scalar.activation(yo[:, 0:512], yt_ps[:, 0:512],
                                         AFT.Identity, bias=nb, scale=sc)
                    nc.vector.tensor_scalar(
                        out=yo[:, 512:1024], in0=yt_ps[:, 512:1024],
                        scalar1=sc, scalar2=nb,
                        op0=mybir.AluOpType.mult,
                        op1=mybir.AluOpType.add)
                    if k % 2 == 0:
                        nc.sync.dma_start(out=y_d[b][:, sl], in_=yo)
                    else:
                        nc.gpsimd.dma_start(out=y_d[b][:, sl], in_=yo)

    nc.compile()
    return nc


_NC_CACHE = None


def _get_nc():
    global _NC_CACHE
    if _NC_CACHE is None:
        _NC_CACHE = build_program()
    return _NC_CACHE


def make_in_maps(x, wq, wk, wv, wo, gamma, beta):
    import ml_dtypes
    f = np.float32
    bf = ml_dtypes.bfloat16
    ones1 = np.ones((1, N), f).astype(bf)
    in_maps = []
    for h in range(H):
        cs = slice(h * C, (h + 1) * C)
        xh = np.ascontiguousarray(x[:, cs, :]).astype(bf)
        # x^T tiles with ones column: [B, 128, NT, 33]
        xt = np.ones((B, 128, NT, 33), f)
        xtr = np.ascontiguousarray(x[:, cs, :].transpose(0, 2, 1))  # [B,N,C]
        xt[:, :, :, :32] = xtr.reshape(B, NT, 128, C).transpose(0, 2, 1, 3)
        wall = np.empty((C, 96), f)
        wall[:, 0:32] = -2.0 * (wk[h].T @ wq[h])    # g-rows weight (G.T)
        wall[:, 32:64] = wq[h].T
        wall[:, 64:96] = wk[h].T
        in_maps.append({
            "xh": xh,
            "xt": xt.astype(bf),
            "on1": ones1,
            "wall": wall.astype(bf),
            "w2t": np.ascontiguousarray((wo[h] @ wv[h]).T.astype(f)),
            "gm": np.ascontiguousarray(gamma[cs].reshape(C, 1).astype(f)),
            "bt": np.ascontiguousarray(beta[cs].reshape(C, 1).astype(f)),
        })
    return in_maps


def kernel(x, wq, wk, wv, wo, bo, gamma, beta):
    x, wq, wk, wv, wo, gamma, beta = (np.asarray(a) for a in
                                      (x, wq, wk, wv, wo, gamma, beta))
    nc = _get_nc()
    in_maps = make_in_maps(x, wq, wk, wv, wo, gamma, beta)
    res = run_bass_kernel_spmd(nc, in_maps, list(range(H)))
    y = np.empty((B, DIM, N), np.float32)
    for h in range(H):
        y[:, h * C:(h + 1) * C, :] = res.results[h]["y"]
    return y
